# revision 14
# baseline (speedup 1.0000x reference)
"""Self-contained 2-layer GCN kernel for 8 Trainium2 NeuronCores.

kernel(**inputs) takes the FULL unsharded inputs (x, edge_index, W1, b1,
W2, b2) and returns the full [N, 128] float32 output.

v3 design (staging-minimized, unified pair plan):
- Host stages only ~4 MB/core: sharded dinv-prescaled x^T (bf16), W1/W2,
  ONE unreplicated int16 gather-index stream pair and ONE int16 one-hot
  column-selector table shared by both layers, plus tiny constants. The
  fp8 layer-1 gather table y = (dinv*x)@W1 is computed ON DEVICE from the
  core's own x shard and published with two fp8 AllGathers.
- Table/AG layout is slot-group-major: group a = slots 0-24, group b =
  25-48; row(core c, slot j, off) = c*SA*128 + j*128 + off within its
  group table. Each group is a separate <32768-row table (int16 gather
  indices) and a separate DRAM tensor, so gathers depend only on their
  group's AllGather (tab2's group-a AG is fired mid-layer-1 and fully
  hidden; only the group-b AG is partially exposed).
- BOTH layers aggregate in the transposed orientation, grouped by target
  slot-PAIR (256 targets): psumT[feat, 2*128] += msg^T @ multi-hot. The
  same edge order serves both layers, so the index streams, chunk
  template and csel table are shared. Layer 1 uses fp8 DoubleRow matmuls
  (one PE instruction = two chunk products at 0.5 cycles/row); its
  epilogue needs NO transposes: psumT IS h^T, feeding the dense W2
  matmuls directly.
- Per-pair epilogues are software-pipelined one pair behind aggregation
  so the in-order PE/ACT queues never head-of-line block; the PSUM "agg"
  tag is shared by the y-compute, layer-1 and layer-2 phases.
- Gathers run via dma_gather on 4 SWDGE queues (2 dedicated per group
  stream, deep prefetch on the group-a streams to overlap the AllGather
  windows); self-loops are applied densely from SBUF-resident tiles.
- Output is written transposed (pairT layout) in bf16.
"""
import numpy as np
import ml_dtypes

import jax
import jax.numpy as jnp
from jax.sharding import Mesh, PartitionSpec
from jax.experimental.shard_map import shard_map

import concourse.bacc as bacc
import concourse.mybir as mybir
import concourse.tile as tile
from concourse.bass2jax import _bass_exec_p, install_neuronx_cc_hook, partition_id_tensor

P = 128
F32 = mybir.dt.float32
BF16 = mybir.dt.bfloat16
FP8 = mybir.dt.float8e4
I16 = mybir.dt.int16
NP_BF16 = ml_dtypes.bfloat16
NP_FP8 = ml_dtypes.float8_e4m3


# ----------------------------------------------------------------------------
# Host-side planning
# ----------------------------------------------------------------------------

def _pack_idx(vals: np.ndarray) -> np.ndarray:
    """Pack an int16 index stream into the UNREPLICATED [16, n/16] layout.

    Position i is read from idxs[i % 16, i // 16]; the device replicates the
    16-row pattern 8x down the partitions (one copy per Q7 core).
    """
    n = len(vals)
    assert n % 16 == 0
    if n == 0:
        return np.zeros((16, 0), np.int16)
    arr16 = np.asarray(vals, np.int16).reshape(n // 16, 16).T  # [16, n/16]
    return np.ascontiguousarray(arr16)


class LayerPlan:
    """Pair-grouped gather/one-hot plan shared by both layers.

    e_idx: per-edge local row within its group table; e_half: 0 (group a)
    or 1 (group b); groups edges by (tgt_core, tgt_pair, e_half).
    """

    def __init__(self, e_idx, e_half, tgt_core, tgt_grp, tgt_off, n_cores,
                 n_grps):
        assert e_idx.max() < 2 ** 15
        key = ((tgt_core * n_grps + tgt_grp) * 2 + e_half)
        order = np.argsort(key, kind="stable")
        idx_s = e_idx[order]
        off_s = tgt_off[order]
        n_groups = n_cores * n_grps * 2
        counts = np.bincount(key[order], minlength=n_groups).reshape(
            n_cores, n_grps, 2)
        chunks = -(-counts // P)  # ceil div
        self.K = chunks.max(axis=0)  # [n_grps, 2] template (max over cores)
        starts = np.concatenate([[0], np.cumsum(counts.reshape(-1))])
        self.idx_streams = []   # per core: (a_vals, b_vals)
        self.csel = []          # per core: [128, n_chunks] int16 (-1 pad)
        nch = int(self.K.sum())
        self.n_chunks = nch
        for c in range(n_cores):
            a_parts, b_parts = [], []
            cs = np.full((nch, P), -1, np.int16)
            ck = 0
            for j in range(n_grps):
                for h in range(2):
                    g = (c * n_grps + j) * 2 + h
                    cnt = counts[c, j, h]
                    kk = int(self.K[j, h])
                    vals = np.zeros(kk * P, np.int64)
                    sel = np.full(kk * P, -1, np.int16)
                    vals[:cnt] = idx_s[starts[g]:starts[g] + cnt]
                    sel[:cnt] = off_s[starts[g]:starts[g] + cnt]
                    (a_parts if h == 0 else b_parts).append(vals)
                    cs[ck:ck + kk] = sel.reshape(kk, P)
                    ck += kk
            a = np.concatenate(a_parts) if a_parts else np.zeros(0, np.int64)
            b = np.concatenate(b_parts) if b_parts else np.zeros(0, np.int64)
            self.idx_streams.append((a, b))
            self.csel.append(cs.T.copy())  # [128, n_chunks]
        self.tot = (int(self.K[:, 0].sum()) * P, int(self.K[:, 1].sum()) * P)


def plan_host(x, edge_index, W1, b1, W2, b2, n_cores=8, oh_batch2=8,
              refine=(20000, 10000)):
    N, F1 = x.shape
    F2 = W2.shape[1]
    row = np.asarray(edge_index[0], np.int64)
    col = np.asarray(edge_index[1], np.int64)
    assert W1.shape[0] == W1.shape[1] == F1, "W1-folded table needs square W1"

    nb = -(-N // P)
    nbp = -(-nb // n_cores) * n_cores          # padded #blocks (392)
    slots = nbp // n_cores                     # 49
    npad = nbp * P                             # 50176
    SA = (slots + 1) // 2                      # 25 group-a slots
    SB = slots - SA                            # 24 group-b slots
    RA = n_cores * SA * P                      # 25600 rows in table a
    RB = n_cores * SB * P                      # 24576 rows in table b
    assert RA < 2 ** 15 and RB < 2 ** 15

    deg = np.bincount(col, minlength=N).astype(np.float64) + 1.0
    dinv = (deg ** -0.5).astype(np.float32)

    srcs = row
    tgts = col
    blk = tgts // P
    sblk = srcs // P

    # --- balance target blocks across (core, slot) ---
    # Two refine rounds: round 1 labels edges by a proxy split (original id
    # halves), round 2 by the actual slot-group membership from round 1.
    tot_cnt = np.bincount(blk, minlength=nbp)
    order = np.argsort(-(-(-tot_cnt // P)), kind="stable")
    assign = order.reshape(slots, n_cores).copy()  # assign[j, c] = block id
    rng_ = np.random.default_rng(0)
    half0 = npad // 2
    for rnd, iters in enumerate(refine):
        if rnd == 0:
            e_lab = (srcs >= half0).astype(np.int64)
        else:
            slot_of_blk = np.empty(nbp, np.int64)
            for j in range(slots):
                slot_of_blk[assign[j]] = j
            e_lab = (slot_of_blk[sblk] >= SA).astype(np.int64)
        a_cnt = np.bincount(blk[e_lab == 0], minlength=nbp)
        b_cnt = np.bincount(blk[e_lab == 1], minlength=nbp)
        a_ch = -(-a_cnt // P)
        b_ch = -(-b_cnt // P)

        def band_cost(band):
            return a_ch[band].max() + b_ch[band].max()

        costs = np.array([band_cost(assign[j]) for j in range(slots)])
        for _ in range(iters):
            j1, j2 = rng_.integers(0, slots, 2)
            if j1 == j2:
                continue
            c1, c2 = rng_.integers(0, n_cores, 2)
            b1_, b2_ = assign[j1, c1], assign[j2, c2]
            assign[j1, c1], assign[j2, c2] = b2_, b1_
            n1, n2 = band_cost(assign[j1]), band_cost(assign[j2])
            if n1 + n2 < costs[j1] + costs[j2]:
                costs[j1], costs[j2] = n1, n2
            else:
                assign[j1, c1], assign[j2, c2] = b1_, b2_

    core_of_blk = np.empty(nbp, np.int64)
    slot_of_blk = np.empty(nbp, np.int64)
    for j in range(slots):
        for c in range(n_cores):
            b = assign[j, c]
            core_of_blk[b] = c
            slot_of_blk[b] = j

    # local row within the block's group table
    grp_of_blk = (slot_of_blk >= SA).astype(np.int64)
    lrow_base = np.where(
        grp_of_blk == 0,
        core_of_blk * SA * P + slot_of_blk * P,
        core_of_blk * SB * P + (slot_of_blk - SA) * P,
    )

    e_half = grp_of_blk[sblk]
    e_idx = lrow_base[sblk] + (srcs % P)
    tgt_core = core_of_blk[blk]
    tgt_slot = slot_of_blk[blk]
    tgt_off = (tgts % P).astype(np.int64)

    npairs = -(-slots // 2)
    off2 = (tgt_slot % 2) * P + tgt_off
    lp = LayerPlan(e_idx, e_half, tgt_core, tgt_slot // 2, off2,
                   n_cores, npairs)

    # --- per-core staged tensors ---
    dinv_pad = np.zeros(npad, np.float32)
    dinv_pad[:N] = dinv
    xpad = np.zeros((npad, F1), np.float32)
    xpad[:N] = np.asarray(x, np.float32) * dinv[:, None]

    w1s = np.ascontiguousarray(
        np.asarray(W1, np.float32).reshape(2, P, F1).transpose(1, 0, 2)
    ).astype(NP_BF16)
    w2s = np.ascontiguousarray(
        np.asarray(W2, np.float32).reshape(2, P, F2).transpose(1, 0, 2)
    ).astype(NP_BF16)
    iot = np.tile(np.arange(2 * P, dtype=np.float32), (P, 1)).astype(NP_BF16)
    ident = np.eye(P, dtype=np.float32).astype(NP_BF16)

    b1_zero = bool(np.all(np.asarray(b1) == 0))
    b2_zero = bool(np.all(np.asarray(b2) == 0))

    in_maps = []
    for c in range(n_cores):
        nodes = (assign[:, c][:, None] * P + np.arange(P)[None, :]).reshape(-1)
        xt = xpad[nodes]                       # [6272, F1] slot-major
        xT = np.ascontiguousarray(
            xt.T.reshape(2, P, slots * P).transpose(1, 0, 2)
        ).astype(NP_BF16)                      # [128, 2, 6272]
        d2 = np.zeros((1, npairs * 2 * P), np.float32)
        d2[0, :slots * P] = dinv_pad[nodes]
        m = {
            "xT": xT,
            "w1": w1s,
            "w2": w2s,
            "iot": iot,
            "ident": ident,
            "dtgt": dinv_pad[assign[:, c][:, None] * P
                             + np.arange(P)[None, :]].T.copy(),  # [128, slots]
            "d2": d2.astype(NP_BF16),
            "cs": lp.csel[c],                  # int16 (offsets reach 255)
            "idxa": _pack_idx(lp.idx_streams[c][0]),
            "idxb": _pack_idx(lp.idx_streams[c][1]),
        }
        if not b1_zero:
            m["b1c"] = np.tile(np.asarray(b1, np.float32).reshape(2, P, 1),
                               (1, 1, 1)).transpose(1, 0, 2).copy()  # [P,2,1]
        if not b2_zero:
            m["b2c"] = np.asarray(b2, np.float32).reshape(P, 1).copy()
        in_maps.append(m)

    meta = {
        "N": N, "F1": F1, "F2": F2, "n_cores": n_cores,
        "b1_zero": b1_zero, "b2_zero": b2_zero,
        "slots": slots, "npad": npad, "SA": SA, "SB": SB, "RA": RA, "RB": RB,
        "K": lp.K, "tot": lp.tot, "nch": lp.n_chunks,
        "assign": assign,
        "npairs": npairs, "oh_batch2": oh_batch2,
    }
    return in_maps, meta


def assemble_output(shards, meta):
    """shards: per core [F2, npairs*256] (transposed pairT layout)
    -> full [N, F2] float32."""
    n_cores, slots = meta["n_cores"], meta["slots"]
    F2, N, npad = meta["F2"], meta["N"], meta["npad"]
    assign = meta["assign"]
    out = np.empty((npad, F2), np.float32)
    for c in range(n_cores):
        sh = np.asarray(shards[c], np.float32)  # [F2, npairs*256]
        for j in range(slots):
            mI, r = divmod(j, 2)
            col0 = mI * 2 * P + r * P
            b = assign[j, c]
            out[b * P:(b + 1) * P] = sh[:, col0:col0 + P].T
    return out[:N]


# ----------------------------------------------------------------------------
# Device program
# ----------------------------------------------------------------------------

class GatherStream:
    """Issues batched dma_gathers for one (group, layer) idx stream and
    hands out per-chunk rhs APs. Slabs round-robin across SWDGE queues."""

    def __init__(self, nc, pool, table_ap, idx_tile, total_idx, feat, tag,
                 slab_chunks=16, bufs=4, queues=(0,), dt=BF16):
        self.nc = nc
        self.dt = dt
        self.pool = pool
        self.table_ap = table_ap
        self.idx_tile = idx_tile
        self.total = total_idx
        self.feat = feat
        self.tag = tag
        self.slab = slab_chunks
        self.bufs = bufs
        self.queues = queues
        self.pos = 0              # chunk cursor
        self.cur_tile = None

    def _ensure(self, skip_gather):
        s, c = divmod(self.pos, self.slab)
        if c == 0:
            if skip_gather:
                if self.cur_tile is None:
                    t = self.pool.tile([P, self.slab, self.feat], self.dt,
                                       tag=self.tag + "z", bufs=1)
                    self.nc.vector.memset(t[:], 0.0)
                    self.cur_tile = t
                return
            base = s * self.slab * P
            n_idx = min(self.slab * P, self.total - base)
            k = n_idx // P
            t = self.pool.tile([P, self.slab, self.feat], self.dt,
                               tag=self.tag, bufs=self.bufs)
            self.nc.gpsimd.dma_gather(
                out_ap=t[:, :k, :],
                in_ap=self.table_ap,
                idxs_ap=self.idx_tile[:, base // 16:(base + n_idx) // 16],
                num_idxs=n_idx,
                num_idxs_reg=n_idx,
                elem_size=self.feat,
                single_packet=False,
                queue_num=self.queues[s % len(self.queues)],
            )
            self.cur_tile = t

    def can_pair(self):
        return self.pos % self.slab != self.slab - 1

    def next_chunk(self, skip_gather=False):
        self._ensure(skip_gather)
        c = self.pos % self.slab
        self.pos += 1
        return self.cur_tile[:, c, :]

    def next_chunk2(self, skip_gather=False):
        self._ensure(skip_gather)
        c = self.pos % self.slab
        assert c != self.slab - 1
        self.pos += 2
        return self.cur_tile[:, c:c + 2, :]

    def take1(self, skip_gather=False):
        """Advance one chunk; returns (tile, offset) for sliced views."""
        self._ensure(skip_gather)
        c = self.pos % self.slab
        self.pos += 1
        return self.cur_tile, c

    def take2(self, skip_gather=False):
        """Advance two chunks (same slab); returns (tile, offset)."""
        self._ensure(skip_gather)
        c = self.pos % self.slab
        assert c != self.slab - 1
        self.pos += 2
        return self.cur_tile, c


def build_nc(meta, slab_chunks=16, n_cores=None, collective=True, io_only=False,
             oh_batch=8, dma_scratch=32768, n_queues=4, mode="full",
             repeat=1, agg_bufs=4, double_row=True, pipe=True,
             gbufs=4, gbufs1a=8, gbufs2a=6):
    n_cores = n_cores or meta["n_cores"]
    slots, SA, SB = meta["slots"], meta["SA"], meta["SB"]
    RA, RB = meta["RA"], meta["RB"]
    F1, F2 = meta["F1"], meta["F2"]
    K = meta["K"]
    nch = meta["nch"]
    npairs = meta["npairs"]
    OB = oh_batch
    nsh = slots * P
    b1_zero = meta["b1_zero"]
    b2_zero = meta["b2_zero"]
    DR = mybir.MatmulPerfMode.DoubleRow

    nc = bacc.Bacc(num_devices=n_cores, dynamic_dma_scratch_size=dma_scratch,
                   num_swdge_queues=n_queues)
    dp = nc.declare_dram_parameter
    xT = dp("xT", [P, 2, nsh], BF16, isOutput=False)
    w1 = dp("w1", [P, 2, F1], BF16, isOutput=False)
    w2 = dp("w2", [P, 2, F2], BF16, isOutput=False)
    iot = dp("iot", [P, 2 * P], BF16, isOutput=False)
    ident = dp("ident", [P, P], BF16, isOutput=False)
    dtgt = dp("dtgt", [P, slots], F32, isOutput=False)
    d2 = dp("d2", [1, npairs * 2 * P], BF16, isOutput=False)
    cs = dp("cs", [P, nch], I16, isOutput=False)
    idxa = dp("idxa", [16, meta["tot"][0] // 16], I16, isOutput=False)
    idxb = dp("idxb", [16, meta["tot"][1] // 16], I16, isOutput=False)
    if not b1_zero:
        b1c = dp("b1c", [P, 2, 1], F32, isOutput=False)
    if not b2_zero:
        b2c = dp("b2c", [P, 1], F32, isOutput=False)
    tick = dp("tick", [1, 4], F32, isOutput=False)
    out = dp("out", [F2, npairs * 2 * P], BF16, isOutput=True)
    tock = dp("tock", [1, 4], F32, isOutput=True)

    # AG shards + gather tables (separate tensors per group => gathers from
    # group a depend only on group a's AllGather)
    ysh_a = nc.dram_tensor("ysh_a", [SA * P, F1], FP8)
    ysh_b = nc.dram_tensor("ysh_b", [SB * P, F1], FP8)
    xs_a = nc.dram_tensor("xs_a", [RA, F1], FP8, addr_space="Shared")
    xs_b = nc.dram_tensor("xs_b", [RB, F1], FP8, addr_space="Shared")
    m2h_a = nc.dram_tensor("m2h_a", [SA * P, F2], BF16)
    m2h_b = nc.dram_tensor("m2h_b", [SB * P, F2], BF16)
    t2_a = nc.dram_tensor("t2_a", [RA, F2], BF16, addr_space="Shared")
    t2_b = nc.dram_tensor("t2_b", [RB, F2], BF16, addr_space="Shared")

    AL = mybir.AluOpType
    ACT = mybir.ActivationFunctionType

    def fire_ag(src, dst):
        if collective:
            nc.gpsimd.collective_compute(
                "AllGather", AL.bypass,
                replica_groups=[list(range(n_cores))],
                ins=[src.ap().opt()], outs=[dst[:, :].opt()],
            )
        else:
            nc.sync.dma_start(dst[0:src.shape[0], :], src[:, :])

    with tile.TileContext(nc) as tc:
        with (
            tc.tile_pool(name="const", bufs=1) as cpool,
            tc.tile_pool(name="msg", bufs=2) as mpool,
            tc.tile_pool(name="work", bufs=2) as wpool,
            tc.tile_pool(name="psum", bufs=2, space="PSUM") as ppool,
        ):
            # timing passthrough: tock = tick (chained-repeat measurement)
            tick_t = cpool.tile([1, 4], F32, tag="tick", bufs=1)
            nc.sync.dma_start(tick_t[:], tick[:, :])
            nc.sync.dma_start(tock[:, :], tick_t[:])

            def load_const(ap, shape, dtype, name):
                t = cpool.tile(shape, dtype, tag=name, bufs=1)
                nc.sync.dma_start(t[:], ap)
                return t

            w1_t = load_const(w1[:, :, :], [P, 2, F1], BF16, "w1")
            w2_t = load_const(w2[:, :, :], [P, 2, F2], BF16, "w2")
            iot_t = load_const(iot[:, :], [P, 2 * P], BF16, "iot")
            id_t = load_const(ident[:, :], [P, P], BF16, "ident")
            dt_t = load_const(dtgt[:, :], [P, slots], F32, "dtgt")
            if not b1_zero:
                b1_t = load_const(b1c[:, :, :], [P, 2, 1], F32, "b1")
            if not b2_zero:
                b2_t = load_const(b2c[:, :], [P, 1], F32, "b2")

            # io2 = tile(arange(256), OB) -- built from iot by doubling
            io2_t = cpool.tile([P, OB * 2 * P], BF16, tag="io2", bufs=1)
            nc.vector.tensor_copy(io2_t[:, 0:2 * P], iot_t[:, :])
            w = 2 * P
            while w < OB * 2 * P:
                n = min(w, OB * 2 * P - w)
                nc.vector.tensor_copy(io2_t[:, w:w + n], io2_t[:, 0:n])
                w += n

            # d2 broadcast row -> full tile
            d2row_t = load_const(d2[:, :], [1, npairs * 2 * P], BF16, "d2r")
            d2_t = cpool.tile([P, npairs * 2 * P], BF16, tag="d2", bufs=1)
            nc.gpsimd.partition_broadcast(d2_t[:], d2row_t[:])

            # csel int16 -> bf16
            cs_16 = wpool.tile([P, nch], I16, tag="cs16", bufs=1)
            nc.sync.dma_start(cs_16[:], cs[:, :])
            cs_t = cpool.tile([P, nch], BF16, tag="cs", bufs=1)
            nc.vector.tensor_copy(cs_t[:], cs_16[:])

            # idx streams: load [16, X] then replicate 8x down partitions
            def load_idx(ap, total, name):
                t = cpool.tile([P, total // 16], I16, tag=name, bufs=1)
                for s in range(8):
                    nc.sync.dma_start(t[16 * s:16 * (s + 1), :], ap)
                return t

            ia_t = load_idx(idxa[:, :], meta["tot"][0], "ixa")
            ib_t = load_idx(idxb[:, :], meta["tot"][1], "ixb")

            # persistent per-slot tiles
            ySelf = cpool.tile([P, slots, F1], FP8, tag="ySelf", bufs=1)
            xw2s = cpool.tile([P, slots, F2], BF16, tag="xw2s", bufs=1)

            qa = tuple(range(n_queues // 2))
            qb = tuple(range(n_queues // 2, n_queues))

            skip_g = mode == "no_gather"
            skip_mm = mode == "gather_only"

            if skip_mm or io_only:
                zo = wpool.tile([P, 2 * P], BF16, tag="outz", bufs=1)
                nc.vector.memset(zo[:], 0.0)
                z2 = wpool.tile([P, F2], BF16, tag="m2z", bufs=1)
                nc.vector.memset(z2[:], 0.0)
            if skip_mm:
                nc.vector.memset(ySelf[:], 0.0)

            if io_only:
                for m_ in range(npairs):
                    nc.sync.dma_start(out[:, m_ * 2 * P:(m_ + 1) * 2 * P],
                                      zo[:])

            for _rep in range(repeat):
              if _rep > 0:
                  # full barrier so R-diff timing measures serial per-pass
                  # time (matches back-to-back single executions)
                  tc.strict_bb_all_engine_barrier()

              # ---- stage 1: y = x~ @ W1 (fp8) + publish via 2 AllGathers --
              if not io_only:
                for j in range(slots):
                    xsl = wpool.tile([P, 2, P], BF16, tag="xsl", bufs=3)
                    nc.sync.dma_start(xsl[:], xT[:, :, j * P:(j + 1) * P])
                    if not skip_mm:
                        py = ppool.tile([P, F1], F32, tag="agg", bufs=agg_bufs)
                        for k in range(2):
                            nc.tensor.matmul(py[:], lhsT=xsl[:, k, :],
                                             rhs=w1_t[:, k, :],
                                             start=(k == 0), stop=(k == 1))
                        nc.scalar.activation(ySelf[:, j, :], py[:], ACT.Copy)
                    if j < SA:
                        nc.sync.dma_start(ysh_a[j * P:(j + 1) * P, :],
                                          ySelf[:, j, :])
                        if j == SA - 1:
                            fire_ag(ysh_a, xs_a)
                    else:
                        jb = j - SA
                        nc.sync.dma_start(ysh_b[jb * P:(jb + 1) * P, :],
                                          ySelf[:, j, :])
                        if j == slots - 1:
                            fire_ag(ysh_b, xs_b)

              st1 = [
                  GatherStream(nc, mpool, xs_a[:, :], ia_t, meta["tot"][0],
                               F1, "m1a", slab_chunks, bufs=gbufs1a,
                               queues=qa, dt=FP8),
                  GatherStream(nc, mpool, xs_b[:, :], ib_t, meta["tot"][1],
                               F1, "m1b", slab_chunks, bufs=gbufs,
                               queues=qb, dt=FP8),
              ]
              st2 = [
                  GatherStream(nc, mpool, t2_a[:, :], ia_t, meta["tot"][0],
                               F2, "m2a", slab_chunks, bufs=gbufs2a,
                               queues=qa),
                  GatherStream(nc, mpool, t2_b[:, :], ib_t, meta["tot"][1],
                               F2, "m2b", slab_chunks, bufs=gbufs,
                               queues=qb),
              ]

              # ---- stage 2: layer-1 transposed aggregation per pair ----
              def agg_l1(m_, ck):
                  nch_m = int(K[m_, 0] + K[m_, 1])
                  if skip_mm:
                      for h in range(2):
                          for _ in range(int(K[m_, h])):
                              st1[h].next_chunk()
                      return None, ck + nch_m
                  ph = []
                  for f in range(2):
                      pht = ppool.tile([P, 2 * P], F32, tag="agg",
                                       bufs=agg_bufs)
                      ph.append(pht)
                  started = [False, False]
                  for h in range(2):
                      left = int(K[m_, h])
                      while left > 0:
                          nb = min(OB, left)
                          oh = wpool.tile([P, OB, 2 * P], FP8, tag="oh",
                                          bufs=2)
                          nc.vector.tensor_tensor(
                              out=oh[:, :nb, :],
                              in0=cs_t[:, ck:ck + nb, None]
                                  .to_broadcast([P, nb, 2 * P]),
                              in1=io2_t[:, :nb * 2 * P],
                              op=AL.is_equal,
                          )
                          i = 0
                          while i < nb:
                              if (double_row and i + 1 < nb
                                      and st1[h].can_pair()):
                                  mt, c = st1[h].take2(skip_g)
                                  for f in range(2):
                                      nc.tensor.matmul(
                                          ph[f][:],
                                          lhsT=mt[:, c:c + 2,
                                                  f * P:(f + 1) * P],
                                          rhs=oh[:, i:i + 2, :],
                                          start=not started[f], stop=False,
                                          perf_mode=DR)
                                      started[f] = True
                                  i += 2
                              else:
                                  mt, c = st1[h].take1(skip_g)
                                  for f in range(2):
                                      nc.tensor.matmul(
                                          ph[f][:],
                                          lhsT=mt[:, c, f * P:(f + 1) * P],
                                          rhs=oh[:, i, :],
                                          start=not started[f], stop=False)
                                      started[f] = True
                                  i += 1
                          ck += nb
                          left -= nb
                  nsl = min(2, slots - 2 * m_)
                  for k in range(nsl):
                      j = 2 * m_ + k
                      for f in range(2):
                          nc.tensor.matmul(
                              ph[f][:, k * P:(k + 1) * P],
                              lhsT=ySelf[:, j, f * P:(f + 1) * P],
                              rhs=id_t[:],
                              start=not started[f],
                              stop=(k == nsl - 1))
                          started[f] = True
                  return ph, ck

              def epi_l1(m_, ph):
                  nsl = min(2, slots - 2 * m_)
                  cols = slice(m_ * 2 * P, (m_ + 1) * 2 * P)
                  if ph is None:
                      for k in range(nsl):
                          j = 2 * m_ + k
                          if j < SA:
                              nc.sync.dma_start(m2h_a[j * P:(j + 1) * P, :],
                                                z2[:])
                          else:
                              jb = j - SA
                              nc.sync.dma_start(m2h_b[jb * P:(jb + 1) * P, :],
                                                z2[:])
                          if j == SA - 1:
                              fire_ag(m2h_a, t2_a)
                          if j == slots - 1:
                              fire_ag(m2h_b, t2_b)
                      return
                  hT = wpool.tile([P, 2, 2 * P], BF16, tag="hT", bufs=2)
                  for f in range(2):
                      tm = wpool.tile([P, 2 * P], F32, tag="ep1", bufs=2)
                      nc.vector.tensor_tensor(tm[:], ph[f][:], d2_t[:, cols],
                                              op=AL.mult)
                      if not b1_zero:
                          nc.vector.tensor_scalar(tm[:], tm[:],
                                                  b1_t[:, f, 0:1], None,
                                                  op0=AL.add)
                      nc.scalar.activation(hT[:, f, :], tm[:], ACT.Relu)
                  for k in range(nsl):
                      j = 2 * m_ + k
                      pd2 = ppool.tile([P, F2], F32, tag="dense", bufs=2)
                      for f in range(2):
                          nc.tensor.matmul(
                              pd2[:],
                              lhsT=hT[:, f, k * P:(k + 1) * P],
                              rhs=w2_t[:, f, :],
                              start=(f == 0), stop=(f == 1))
                      nc.scalar.activation(xw2s[:, j, :], pd2[:], ACT.Copy,
                                           scale=dt_t[:, j:j + 1])
                      if j < SA:
                          nc.sync.dma_start(m2h_a[j * P:(j + 1) * P, :],
                                            xw2s[:, j, :])
                      else:
                          jb = j - SA
                          nc.sync.dma_start(m2h_b[jb * P:(jb + 1) * P, :],
                                            xw2s[:, j, :])
                      if j == SA - 1:
                          fire_ag(m2h_a, t2_a)
                      if j == slots - 1:
                          fire_ag(m2h_b, t2_b)

              ck1 = 0
              prev = None
              for m_ in range(npairs if not io_only else 0):
                  ph, ck1 = agg_l1(m_, ck1)
                  if prev is not None:
                      epi_l1(*prev)
                  prev = (m_, ph) if pipe else None
                  if not pipe:
                      epi_l1(m_, ph)
              if prev is not None:
                  epi_l1(*prev)

              # ---- stage 3: layer-2 aggregation + epilogue ----
              def agg_l2(m_, ck):
                  nch_m = int(K[m_, 0] + K[m_, 1])
                  if skip_mm:
                      for h in range(2):
                          for _ in range(int(K[m_, h])):
                              st2[h].next_chunk()
                      return None, ck + nch_m
                  psumT = ppool.tile([P, 2 * P], F32, tag="agg",
                                     bufs=agg_bufs)
                  ci = 0
                  for h in range(2):
                      left = int(K[m_, h])
                      while left > 0:
                          nb = min(OB, left)
                          moh = wpool.tile([P, OB, 2 * P], BF16, tag="moh",
                                           bufs=2)
                          nc.vector.tensor_tensor(
                              out=moh[:, :nb, :],
                              in0=cs_t[:, ck:ck + nb, None]
                                  .to_broadcast([P, nb, 2 * P]),
                              in1=io2_t[:, :nb * 2 * P],
                              op=AL.is_equal,
                          )
                          for i in range(nb):
                              msg = st2[h].next_chunk(skip_g)
                              nc.tensor.matmul(psumT[:], lhsT=msg,
                                               rhs=moh[:, i, :],
                                               start=(ci == 0), stop=False)
                              ci += 1
                          ck += nb
                          left -= nb
                  nsl = min(2, slots - 2 * m_)
                  for k in range(nsl):
                      nc.tensor.matmul(psumT[:, k * P:(k + 1) * P],
                                       lhsT=xw2s[:, 2 * m_ + k, :],
                                       rhs=id_t[:],
                                       start=(nch_m == 0 and k == 0),
                                       stop=(k == nsl - 1))
                  return psumT, ck

              def epi_l2(m_, psumT):
                  if psumT is None:
                      nc.sync.dma_start(out[:, m_ * 2 * P:(m_ + 1) * 2 * P],
                                        zo[:])
                      return
                  t2 = wpool.tile([P, 2 * P], BF16, tag="ep2", bufs=2)
                  if b2_zero:
                      nc.vector.tensor_tensor(
                          t2[:], psumT[:],
                          d2_t[:, m_ * 2 * P:(m_ + 1) * 2 * P], op=AL.mult)
                  else:
                      tf = wpool.tile([P, 2 * P], F32, tag="ep2f", bufs=2)
                      nc.vector.tensor_tensor(
                          tf[:], psumT[:],
                          d2_t[:, m_ * 2 * P:(m_ + 1) * 2 * P], op=AL.mult)
                      nc.vector.tensor_scalar(t2[:], tf[:], b2_t[:, 0:1],
                                              None, op0=AL.add)
                  nc.sync.dma_start(out[:, m_ * 2 * P:(m_ + 1) * 2 * P],
                                    t2[:])

              ck2 = 0
              prev2 = None
              for m_ in range(npairs if not io_only else 0):
                  psumT, ck2 = agg_l2(m_, ck2)
                  if prev2 is not None:
                      epi_l2(*prev2)
                  prev2 = (m_, psumT) if pipe else None
                  if not pipe:
                      epi_l2(m_, psumT)
              if prev2 is not None:
                  epi_l2(*prev2)

    nc.compile()
    return nc


class SpmdRunner:
    def __init__(self, nc, n_cores: int = 8, nreps: int = 1,
                 tick_name: str = "tick", tock_name: str = "tock"):
        install_neuronx_cc_hook()
        self.nc = nc
        self.n_cores = n_cores
        assert nc.dbg_addr is None or not nc.dbg_callbacks
        self.dbg_name = nc.dbg_addr.name if nc.dbg_addr is not None else None
        partition_name = nc.partition_id_tensor.name if nc.partition_id_tensor else None

        in_names, out_names, out_avals = [], [], []
        for alloc in nc.m.functions[0].allocations:
            if not isinstance(alloc, mybir.MemoryLocationSet):
                continue
            name = alloc.memorylocations[0].name
            if alloc.kind == "ExternalInput":
                if name != partition_name:
                    in_names.append(name)
            elif alloc.kind == "ExternalOutput":
                out_names.append(name)
                shape = tuple(alloc.tensor_shape)
                dtype = mybir.dt.np(alloc.dtype)
                out_avals.append(jax.core.ShapedArray(shape, dtype))
        self.in_names = in_names      # order matters; includes dbg if declared
        self.out_names = out_names
        self.out_avals = out_avals
        n_params = len(in_names)
        n_outs = len(out_avals)
        all_in_names = list(in_names) + list(out_names)
        if partition_name is not None:
            all_in_names.append(partition_name)

        tick_i = in_names.index(tick_name) if (nreps > 1 and tick_name in in_names) else None
        tock_i = out_names.index(tock_name) if (nreps > 1 and tock_name in out_names) else None
        assert nreps == 1 or (tick_i is not None and tock_i is not None), \
            "nreps>1 needs tick/tock passthrough tensors in the kernel"

        def _call(operands):
            if partition_name is not None:
                operands = operands + [partition_id_tensor()]
            return _bass_exec_p.bind(
                *operands,
                out_avals=tuple(out_avals),
                in_names=tuple(all_in_names),
                out_names=tuple(out_names),
                lowering_input_output_aliases=(),
                sim_require_finite=True,
                sim_require_nnan=True,
                nc=nc,
            )

        def _body(*args):
            operands = list(args)
            outs = _call(list(operands))
            for _ in range(nreps - 1):
                operands2 = list(operands)
                operands2[tick_i] = outs[tock_i]
                outs = _call(operands2)
            return tuple(outs)

        devices = jax.devices()[: self.n_cores]
        assert len(devices) == self.n_cores
        mesh = Mesh(np.asarray(devices), ("core",))
        self._sharding = jax.sharding.NamedSharding(mesh, PartitionSpec("core"))
        in_specs = (PartitionSpec("core"),) * (n_params + n_outs)
        out_specs = (PartitionSpec("core"),) * n_outs
        self._fn = jax.jit(
            shard_map(_body, mesh=mesh, in_specs=in_specs, out_specs=out_specs,
                      check_rep=False),
            keep_unused=True,
        )
        # output operand buffers are created ON DEVICE (no host staging)
        zero_shapes = [(self.n_cores * z.shape[0], *z.shape[1:])
                       for z in out_avals]
        zero_dtypes = [a.dtype for a in out_avals]

        def _mk_zeros():
            return tuple(jnp.zeros(s, d) for s, d in
                         zip(zero_shapes, zero_dtypes))

        self._zeros_fn = jax.jit(
            _mk_zeros, out_shardings=(self._sharding,) * n_outs)
        self._dev_zeros = None
        self._dev_in = None

    def stage_inputs(self, in_maps):
        """in_maps: list (len n_cores) of dict name->np.ndarray."""
        if self.dbg_name is not None:
            in_maps = [
                {**m, self.dbg_name: np.zeros((1, 2), np.uint32)} for m in in_maps
            ]
        concat_in = [
            np.concatenate([np.asarray(in_maps[c][name]) for c in range(self.n_cores)],
                           axis=0)
            for name in self.in_names
        ]
        self._dev_in = [jax.device_put(a, self._sharding) for a in concat_in]
        self._dev_zeros = list(self._zeros_fn())
        jax.block_until_ready(self._dev_in)
        jax.block_until_ready(self._dev_zeros)

    def run(self):
        outs = self._fn(*self._dev_in, *self._dev_zeros)
        jax.block_until_ready(outs)
        return outs

    def run_chain(self, n):
        """Dispatch n executions back-to-back (tick chained through tock to
        force strict ordering), block once at the end."""
        ti = self.in_names.index("tick")
        oi = self.out_names.index("tock")
        ins = list(self._dev_in)
        outs = self._fn(*ins, *self._dev_zeros)
        for _ in range(n - 1):
            ins[ti] = outs[oi]
            outs = self._fn(*ins, *self._dev_zeros)
        jax.block_until_ready(outs)
        return outs

    def results(self, outs):
        return [
            {
                name: np.asarray(outs[i]).reshape(self.n_cores, *self.out_avals[i].shape)[c]
                for i, name in enumerate(self.out_names)
            }
            for c in range(self.n_cores)
        ]


# ----------------------------------------------------------------------------
# Public entry point
# ----------------------------------------------------------------------------

_CACHE = {}

BUILD_KW = dict(slab_chunks=16, oh_batch=8, agg_bufs=4,
                gbufs=4, gbufs1a=8, gbufs2a=6)


def kernel(**inputs) -> np.ndarray:
    x = np.asarray(inputs["x"], np.float32)
    edge_index = np.asarray(inputs["edge_index"], np.int64)
    W1 = np.asarray(inputs["W1"], np.float32)
    b1 = np.asarray(inputs["b1"], np.float32)
    W2 = np.asarray(inputs["W2"], np.float32)
    b2 = np.asarray(inputs["b2"], np.float32)

    in_maps, meta = plan_host(x, edge_index, W1, b1, W2, b2)
    for m in in_maps:
        m["tick"] = np.zeros((1, 4), np.float32)

    key = (x.shape, edge_index.shape, W2.shape,
           tuple(meta["K"].reshape(-1)),
           meta["b1_zero"], meta["b2_zero"])
    if key not in _CACHE:
        nc = build_nc(meta, **BUILD_KW)
        _CACHE[key] = SpmdRunner(nc, meta["n_cores"])
    runner = _CACHE[key]
    runner.stage_inputs(in_maps)
    outs = runner.run()
    res = runner.results(outs)
    shards = [res[c]["out"] for c in range(meta["n_cores"])]
    return assemble_output(shards, meta).astype(np.float32)


# revision 16
# speedup vs baseline: 2.1964x; 2.1964x over previous
"""Self-contained 2-layer GCN kernel for 8 Trainium2 NeuronCores.

kernel(**inputs) takes the FULL unsharded inputs (x, edge_index, W1, b1,
W2, b2) and returns the full [N, 128] float32 output.

v2.1 design (staging-minimized):
- Host stages only ~4.2 MB/core: sharded dinv-prescaled x^T (bf16), W1/W2,
  unreplicated int16 gather-index streams, int8/int16 one-hot column
  selectors, and a few tiny constants. The fp8 layer-1 gather table
  y = (dinv*x)@W1 is computed ON DEVICE from the core's own x shard and
  published with two fp8 AllGathers (an earlier version staged the
  12.8 MB table replicated per core).
- Table/AG layout is slot-group-major: group a = slots 0-24, group b =
  25-48; row(core c, slot j, off) = c*SA*128 + j*128 + off within its
  group table. Each group is a separate <32768-row table (int16 gather
  indices) and a separate DRAM tensor, so gathers depend only on their
  group's AllGather (tab2's group-a AG is fired mid-layer-1 and fully
  hidden; only the group-b AG is partially exposed).
- Layer-1 aggregation is row-major per slot with fp8 DoubleRow matmuls
  (one PE instruction = two (one-hot)^T @ msg chunk products at
  0.5 cycles/row). Layer 2 uses the transposed slot-PAIR accumulation:
  psumT[feat, 256] += msg^T @ multi-hot, output written transposed bf16.
- Per-slot/pair epilogues are software-pipelined one step behind
  aggregation so the in-order PE/ACT queues never head-of-line block the
  next accumulation; the PSUM "agg" tag is shared by the y-compute,
  layer-1 and layer-2 phases (PE transposes bridge h to the dense W2).
- Gathers run via dma_gather on 4 SWDGE queues (2 dedicated per group
  stream; the group-a streams get deeper buffering to prefetch through
  the AllGather windows); self-loops are applied densely from
  SBUF-resident tiles.
"""
import numpy as np
import ml_dtypes

import jax
import jax.numpy as jnp
from jax.sharding import Mesh, PartitionSpec
from jax.experimental.shard_map import shard_map

import concourse.bacc as bacc
import concourse.mybir as mybir
import concourse.tile as tile
from concourse.bass2jax import _bass_exec_p, install_neuronx_cc_hook, partition_id_tensor

P = 128
F32 = mybir.dt.float32
BF16 = mybir.dt.bfloat16
FP8 = mybir.dt.float8e4
I16 = mybir.dt.int16
I8 = mybir.dt.int8
NP_BF16 = ml_dtypes.bfloat16
NP_FP8 = ml_dtypes.float8_e4m3


# ----------------------------------------------------------------------------
# Host-side planning
# ----------------------------------------------------------------------------

def _pack_idx(vals: np.ndarray) -> np.ndarray:
    """Pack an int16 index stream into the UNREPLICATED [16, n/16] layout.

    Position i is read from idxs[i % 16, i // 16]; the device replicates the
    16-row pattern 8x down the partitions (one copy per Q7 core).
    """
    n = len(vals)
    assert n % 16 == 0
    if n == 0:
        return np.zeros((16, 0), np.int16)
    arr16 = np.asarray(vals, np.int16).reshape(n // 16, 16).T  # [16, n/16]
    return np.ascontiguousarray(arr16)


class LayerPlan:
    """Per-layer gather/one-hot plan: per-core idx streams + csel + template.

    e_idx: per-edge local row within its group table; e_half: 0 (group a)
    or 1 (group b); groups edges by (tgt_core, tgt_group, e_half).
    """

    def __init__(self, e_idx, e_half, tgt_core, tgt_grp, tgt_off, n_cores,
                 n_grps):
        assert e_idx.max() < 2 ** 15
        key = ((tgt_core * n_grps + tgt_grp) * 2 + e_half)
        order = np.argsort(key, kind="stable")
        idx_s = e_idx[order]
        off_s = tgt_off[order]
        n_groups = n_cores * n_grps * 2
        counts = np.bincount(key[order], minlength=n_groups).reshape(
            n_cores, n_grps, 2)
        chunks = -(-counts // P)  # ceil div
        self.K = chunks.max(axis=0)  # [n_grps, 2] template (max over cores)
        starts = np.concatenate([[0], np.cumsum(counts.reshape(-1))])
        self.idx_streams = []   # per core: (a_vals, b_vals)
        self.csel = []          # per core: [128, n_chunks] (-1 pad)
        nch = int(self.K.sum())
        self.n_chunks = nch
        for c in range(n_cores):
            a_parts, b_parts = [], []
            cs = np.full((nch, P), -1, np.int16)
            ck = 0
            for j in range(n_grps):
                for h in range(2):
                    g = (c * n_grps + j) * 2 + h
                    cnt = counts[c, j, h]
                    kk = int(self.K[j, h])
                    vals = np.zeros(kk * P, np.int64)
                    sel = np.full(kk * P, -1, np.int16)
                    vals[:cnt] = idx_s[starts[g]:starts[g] + cnt]
                    sel[:cnt] = off_s[starts[g]:starts[g] + cnt]
                    (a_parts if h == 0 else b_parts).append(vals)
                    cs[ck:ck + kk] = sel.reshape(kk, P)
                    ck += kk
            a = np.concatenate(a_parts) if a_parts else np.zeros(0, np.int64)
            b = np.concatenate(b_parts) if b_parts else np.zeros(0, np.int64)
            self.idx_streams.append((a, b))
            self.csel.append(cs.T.copy())  # [128, n_chunks]
        self.tot = (int(self.K[:, 0].sum()) * P, int(self.K[:, 1].sum()) * P)


def plan_host(x, edge_index, W1, b1, W2, b2, n_cores=8, oh_batch2=8,
              refine=(20000, 10000)):
    N, F1 = x.shape
    F2 = W2.shape[1]
    row = np.asarray(edge_index[0], np.int64)
    col = np.asarray(edge_index[1], np.int64)
    assert W1.shape[0] == W1.shape[1] == F1, "W1-folded table needs square W1"

    nb = -(-N // P)
    nbp = -(-nb // n_cores) * n_cores          # padded #blocks (392)
    slots = nbp // n_cores                     # 49
    npad = nbp * P                             # 50176
    SA = (slots + 1) // 2                      # 25 group-a slots
    SB = slots - SA                            # 24 group-b slots
    RA = n_cores * SA * P                      # 25600 rows in table a
    RB = n_cores * SB * P                      # 24576 rows in table b
    assert RA < 2 ** 15 and RB < 2 ** 15

    deg = np.bincount(col, minlength=N).astype(np.float64) + 1.0
    dinv = (deg ** -0.5).astype(np.float32)

    srcs = row
    tgts = col
    blk = tgts // P
    sblk = srcs // P

    # --- balance target blocks across (core, slot) ---
    # Two refine rounds: round 1 labels edges by a proxy split (original id
    # halves), round 2 by the actual slot-group membership from round 1.
    tot_cnt = np.bincount(blk, minlength=nbp)
    order = np.argsort(-(-(-tot_cnt // P)), kind="stable")
    assign = order.reshape(slots, n_cores).copy()  # assign[j, c] = block id
    rng_ = np.random.default_rng(0)
    half0 = npad // 2
    for rnd, iters in enumerate(refine):
        if rnd == 0:
            e_lab = (srcs >= half0).astype(np.int64)
        else:
            slot_of_blk = np.empty(nbp, np.int64)
            for j in range(slots):
                slot_of_blk[assign[j]] = j
            e_lab = (slot_of_blk[sblk] >= SA).astype(np.int64)
        a_cnt = np.bincount(blk[e_lab == 0], minlength=nbp)
        b_cnt = np.bincount(blk[e_lab == 1], minlength=nbp)
        a_ch = -(-a_cnt // P)
        b_ch = -(-b_cnt // P)

        def band_cost(band):
            return a_ch[band].max() + b_ch[band].max()

        costs = np.array([band_cost(assign[j]) for j in range(slots)])
        for _ in range(iters):
            j1, j2 = rng_.integers(0, slots, 2)
            if j1 == j2:
                continue
            c1, c2 = rng_.integers(0, n_cores, 2)
            b1_, b2_ = assign[j1, c1], assign[j2, c2]
            assign[j1, c1], assign[j2, c2] = b2_, b1_
            n1, n2 = band_cost(assign[j1]), band_cost(assign[j2])
            if n1 + n2 < costs[j1] + costs[j2]:
                costs[j1], costs[j2] = n1, n2
            else:
                assign[j1, c1], assign[j2, c2] = b1_, b2_

    core_of_blk = np.empty(nbp, np.int64)
    slot_of_blk = np.empty(nbp, np.int64)
    for j in range(slots):
        for c in range(n_cores):
            b = assign[j, c]
            core_of_blk[b] = c
            slot_of_blk[b] = j

    # local row within the block's group table
    grp_of_blk = (slot_of_blk >= SA).astype(np.int64)
    lrow_base = np.where(
        grp_of_blk == 0,
        core_of_blk * SA * P + slot_of_blk * P,
        core_of_blk * SB * P + (slot_of_blk - SA) * P,
    )

    e_half = grp_of_blk[sblk]
    e_idx = lrow_base[sblk] + (srcs % P)
    tgt_core = core_of_blk[blk]
    tgt_slot = slot_of_blk[blk]
    tgt_off = (tgts % P).astype(np.int64)

    l1 = LayerPlan(e_idx, e_half, tgt_core, tgt_slot, tgt_off,
                   n_cores, slots)
    npairs = -(-slots // 2)
    off2 = (tgt_slot % 2) * P + tgt_off
    l2 = LayerPlan(e_idx, e_half, tgt_core, tgt_slot // 2, off2,
                   n_cores, npairs)

    # --- per-core staged tensors ---
    dinv_pad = np.zeros(npad, np.float32)
    dinv_pad[:N] = dinv
    xpad = np.zeros((npad, F1), np.float32)
    xpad[:N] = np.asarray(x, np.float32) * dinv[:, None]

    w1s = np.ascontiguousarray(
        np.asarray(W1, np.float32).reshape(2, P, F1).transpose(1, 0, 2)
    ).astype(NP_BF16)
    w2s = np.ascontiguousarray(
        np.asarray(W2, np.float32).reshape(2, P, F2).transpose(1, 0, 2)
    ).astype(NP_BF16)
    iot = np.tile(np.arange(2 * P, dtype=np.float32), (P, 1)).astype(NP_BF16)
    ident = np.eye(P, dtype=np.float32).astype(NP_BF16)
    id8 = np.eye(P, dtype=np.float32).astype(NP_FP8)

    b1_zero = bool(np.all(np.asarray(b1) == 0))
    b2_zero = bool(np.all(np.asarray(b2) == 0))

    in_maps = []
    for c in range(n_cores):
        nodes = (assign[:, c][:, None] * P + np.arange(P)[None, :]).reshape(-1)
        xt = xpad[nodes]                       # [6272, F1] slot-major
        xT = np.ascontiguousarray(
            xt.T.reshape(2, P, slots * P).transpose(1, 0, 2)
        ).astype(NP_BF16)                      # [128, 2, 6272]
        d2 = np.zeros((1, npairs * 2 * P), np.float32)
        d2[0, :slots * P] = dinv_pad[nodes]
        m = {
            "xT": xT,
            "w1": w1s,
            "w2": w2s,
            "iot": iot,
            "ident": ident,
            "id8": id8,
            "dtgt": dinv_pad[assign[:, c][:, None] * P
                             + np.arange(P)[None, :]].T.copy(),  # [128, slots]
            "d2": d2.astype(NP_BF16),
            "cs1": l1.csel[c].astype(np.int8),
            "cs2": l2.csel[c].astype(np.int16),  # offsets reach 255
            "idx1a": _pack_idx(l1.idx_streams[c][0]),
            "idx1b": _pack_idx(l1.idx_streams[c][1]),
            "idx2a": _pack_idx(l2.idx_streams[c][0]),
            "idx2b": _pack_idx(l2.idx_streams[c][1]),
        }
        if not b1_zero:
            m["b1r"] = np.tile(np.asarray(b1, np.float32), (P, 1))
        if not b2_zero:
            m["b2c"] = np.asarray(b2, np.float32).reshape(P, 1).copy()
        in_maps.append(m)

    meta = {
        "N": N, "F1": F1, "F2": F2, "n_cores": n_cores,
        "b1_zero": b1_zero, "b2_zero": b2_zero,
        "slots": slots, "npad": npad, "SA": SA, "SB": SB, "RA": RA, "RB": RB,
        "K1": l1.K, "K2": l2.K,
        "tot1": l1.tot, "tot2": l2.tot,
        "nch1": l1.n_chunks, "nch2": l2.n_chunks,
        "assign": assign,
        "npairs": npairs, "oh_batch2": oh_batch2,
    }
    return in_maps, meta


def assemble_output(shards, meta):
    """shards: per core [F2, npairs*256] (transposed pairT layout)
    -> full [N, F2] float32."""
    n_cores, slots = meta["n_cores"], meta["slots"]
    F2, N, npad = meta["F2"], meta["N"], meta["npad"]
    assign = meta["assign"]
    out = np.empty((npad, F2), np.float32)
    for c in range(n_cores):
        sh = np.asarray(shards[c], np.float32)  # [F2, npairs*256]
        for j in range(slots):
            mI, r = divmod(j, 2)
            col0 = mI * 2 * P + r * P
            b = assign[j, c]
            out[b * P:(b + 1) * P] = sh[:, col0:col0 + P].T
    return out[:N]


# ----------------------------------------------------------------------------
# Device program
# ----------------------------------------------------------------------------

class GatherStream:
    """Issues batched dma_gathers for one (group, layer) idx stream and
    hands out per-chunk rhs APs. Slabs round-robin across SWDGE queues."""

    def __init__(self, nc, pool, table_ap, idx_tile, total_idx, feat, tag,
                 slab_chunks=16, bufs=4, queues=(0,), dt=BF16):
        self.nc = nc
        self.dt = dt
        self.pool = pool
        self.table_ap = table_ap
        self.idx_tile = idx_tile
        self.total = total_idx
        self.feat = feat
        self.tag = tag
        self.slab = slab_chunks
        self.bufs = bufs
        self.queues = queues
        self.pos = 0              # chunk cursor
        self.cur_tile = None

    def _ensure(self, skip_gather):
        s, c = divmod(self.pos, self.slab)
        if c == 0:
            if skip_gather:
                if self.cur_tile is None:
                    t = self.pool.tile([P, self.slab, self.feat], self.dt,
                                       tag=self.tag + "z", bufs=1)
                    self.nc.vector.memset(t[:], 0.0)
                    self.cur_tile = t
                return
            base = s * self.slab * P
            n_idx = min(self.slab * P, self.total - base)
            k = n_idx // P
            t = self.pool.tile([P, self.slab, self.feat], self.dt,
                               tag=self.tag, bufs=self.bufs)
            self.nc.gpsimd.dma_gather(
                out_ap=t[:, :k, :],
                in_ap=self.table_ap,
                idxs_ap=self.idx_tile[:, base // 16:(base + n_idx) // 16],
                num_idxs=n_idx,
                num_idxs_reg=n_idx,
                elem_size=self.feat,
                single_packet=False,
                queue_num=self.queues[s % len(self.queues)],
            )
            self.cur_tile = t

    def can_pair(self):
        return self.pos % self.slab != self.slab - 1

    def next_chunk(self, skip_gather=False):
        self._ensure(skip_gather)
        c = self.pos % self.slab
        self.pos += 1
        return self.cur_tile[:, c, :]

    def next_chunk2(self, skip_gather=False):
        self._ensure(skip_gather)
        c = self.pos % self.slab
        assert c != self.slab - 1
        self.pos += 2
        return self.cur_tile[:, c:c + 2, :]


def build_nc(meta, slab_chunks=16, n_cores=None, collective=True, io_only=False,
             oh_batch=16, dma_scratch=32768, n_queues=4, mode="full",
             repeat=1, agg_bufs=4, double_row=True, pipe=True,
             gbufs=4, gbufs1a=7, gbufs2a=5):
    n_cores = n_cores or meta["n_cores"]
    slots, SA, SB = meta["slots"], meta["SA"], meta["SB"]
    RA, RB = meta["RA"], meta["RB"]
    F1, F2 = meta["F1"], meta["F2"]
    K1, K2 = meta["K1"], meta["K2"]
    nch1, nch2 = meta["nch1"], meta["nch2"]
    npairs = meta["npairs"]
    OB2 = meta["oh_batch2"]
    nsh = slots * P
    b1_zero = meta["b1_zero"]
    b2_zero = meta["b2_zero"]
    DR = mybir.MatmulPerfMode.DoubleRow

    nc = bacc.Bacc(num_devices=n_cores, dynamic_dma_scratch_size=dma_scratch,
                   num_swdge_queues=n_queues)
    dp = nc.declare_dram_parameter
    xT = dp("xT", [P, 2, nsh], BF16, isOutput=False)
    w1 = dp("w1", [P, 2, F1], BF16, isOutput=False)
    w2 = dp("w2", [P, 2, F2], BF16, isOutput=False)
    iot = dp("iot", [P, 2 * P], BF16, isOutput=False)
    ident = dp("ident", [P, P], BF16, isOutput=False)
    id8 = dp("id8", [P, P], FP8, isOutput=False)
    dtgt = dp("dtgt", [P, slots], F32, isOutput=False)
    d2 = dp("d2", [1, npairs * 2 * P], BF16, isOutput=False)
    cs1 = dp("cs1", [P, nch1], I8, isOutput=False)
    cs2 = dp("cs2", [P, nch2], I16, isOutput=False)
    idx1a = dp("idx1a", [16, meta["tot1"][0] // 16], I16, isOutput=False)
    idx1b = dp("idx1b", [16, meta["tot1"][1] // 16], I16, isOutput=False)
    idx2a = dp("idx2a", [16, meta["tot2"][0] // 16], I16, isOutput=False)
    idx2b = dp("idx2b", [16, meta["tot2"][1] // 16], I16, isOutput=False)
    if not b1_zero:
        b1r = dp("b1r", [P, F1], F32, isOutput=False)
    if not b2_zero:
        b2c = dp("b2c", [P, 1], F32, isOutput=False)
    tick = dp("tick", [1, 4], F32, isOutput=False)
    out = dp("out", [F2, npairs * 2 * P], BF16, isOutput=True)
    tock = dp("tock", [1, 4], F32, isOutput=True)

    # AG shards + gather tables (separate tensors per group => gathers from
    # group a depend only on group a's AllGather)
    ysh_a = nc.dram_tensor("ysh_a", [SA * P, F1], FP8)
    ysh_b = nc.dram_tensor("ysh_b", [SB * P, F1], FP8)
    xs_a = nc.dram_tensor("xs_a", [RA, F1], FP8, addr_space="Shared")
    xs_b = nc.dram_tensor("xs_b", [RB, F1], FP8, addr_space="Shared")
    m2h_a = nc.dram_tensor("m2h_a", [SA * P, F2], BF16)
    m2h_b = nc.dram_tensor("m2h_b", [SB * P, F2], BF16)
    t2_a = nc.dram_tensor("t2_a", [RA, F2], BF16, addr_space="Shared")
    t2_b = nc.dram_tensor("t2_b", [RB, F2], BF16, addr_space="Shared")

    AL = mybir.AluOpType
    ACT = mybir.ActivationFunctionType

    def fire_ag(src, dst):
        if collective:
            nc.gpsimd.collective_compute(
                "AllGather", AL.bypass,
                replica_groups=[list(range(n_cores))],
                ins=[src.ap().opt()], outs=[dst[:, :].opt()],
            )
        else:
            nc.sync.dma_start(dst[0:src.shape[0], :], src[:, :])

    with tile.TileContext(nc) as tc:
        with (
            tc.tile_pool(name="const", bufs=1) as cpool,
            tc.tile_pool(name="msg", bufs=2) as mpool,
            tc.tile_pool(name="work", bufs=2) as wpool,
            tc.tile_pool(name="psum", bufs=2, space="PSUM") as ppool,
        ):
            # timing passthrough: tock = tick (chained-repeat measurement)
            tick_t = cpool.tile([1, 4], F32, tag="tick", bufs=1)
            nc.sync.dma_start(tick_t[:], tick[:, :])
            nc.sync.dma_start(tock[:, :], tick_t[:])

            def load_const(ap, shape, dtype, name):
                t = cpool.tile(shape, dtype, tag=name, bufs=1)
                nc.sync.dma_start(t[:], ap)
                return t

            w1_t = load_const(w1[:, :, :], [P, 2, F1], BF16, "w1")
            w2_t = load_const(w2[:, :, :], [P, 2, F2], BF16, "w2")
            iot_t = load_const(iot[:, :], [P, 2 * P], BF16, "iot")
            id_t = load_const(ident[:, :], [P, P], BF16, "ident")
            id8_t = load_const(id8[:, :], [P, P], FP8, "id8")
            dt_t = load_const(dtgt[:, :], [P, slots], F32, "dtgt")
            if not b1_zero:
                b1_t = load_const(b1r[:, :], [P, F1], F32, "b1")
            if not b2_zero:
                b2_t = load_const(b2c[:, :], [P, 1], F32, "b2")

            # one-hot compare sources: io1 = tile(arange(128), oh_batch),
            # io2 = tile(arange(256), OB2) -- built from iot by doubling
            io1_t = cpool.tile([P, oh_batch * P], BF16, tag="io1", bufs=1)
            nc.vector.tensor_copy(io1_t[:, 0:P], iot_t[:, 0:P])
            w = P
            while w < oh_batch * P:
                n = min(w, oh_batch * P - w)
                nc.vector.tensor_copy(io1_t[:, w:w + n], io1_t[:, 0:n])
                w += n
            io2_t = cpool.tile([P, OB2 * 2 * P], BF16, tag="io2", bufs=1)
            nc.vector.tensor_copy(io2_t[:, 0:2 * P], iot_t[:, :])
            w = 2 * P
            while w < OB2 * 2 * P:
                n = min(w, OB2 * 2 * P - w)
                nc.vector.tensor_copy(io2_t[:, w:w + n], io2_t[:, 0:n])
                w += n

            # d2 broadcast row -> full tile
            d2row_t = load_const(d2[:, :], [1, npairs * 2 * P], BF16, "d2r")
            d2_t = cpool.tile([P, npairs * 2 * P], BF16, tag="d2", bufs=1)
            nc.gpsimd.partition_broadcast(d2_t[:], d2row_t[:])

            # csel int8/int16 -> bf16
            cs1_8 = wpool.tile([P, nch1], I8, tag="cs18", bufs=1)
            nc.sync.dma_start(cs1_8[:], cs1[:, :])
            cs1_t = cpool.tile([P, nch1], BF16, tag="cs1", bufs=1)
            nc.vector.tensor_copy(cs1_t[:], cs1_8[:])
            cs2_8 = wpool.tile([P, nch2], I16, tag="cs28", bufs=1)
            nc.sync.dma_start(cs2_8[:], cs2[:, :])
            cs2_t = cpool.tile([P, nch2], BF16, tag="cs2", bufs=1)
            nc.vector.tensor_copy(cs2_t[:], cs2_8[:])

            # idx streams: load [16, X] then replicate 8x down partitions
            def load_idx(ap, total, name):
                t = cpool.tile([P, total // 16], I16, tag=name, bufs=1)
                for s in range(8):
                    nc.sync.dma_start(t[16 * s:16 * (s + 1), :], ap)
                return t

            i1a_t = load_idx(idx1a[:, :], meta["tot1"][0], "ix1a")
            i1b_t = load_idx(idx1b[:, :], meta["tot1"][1], "ix1b")
            i2a_t = load_idx(idx2a[:, :], meta["tot2"][0], "ix2a")
            i2b_t = load_idx(idx2b[:, :], meta["tot2"][1], "ix2b")

            # persistent per-slot tiles
            ySelf = cpool.tile([P, slots, F1], FP8, tag="ySelf", bufs=1)
            xw2s = cpool.tile([P, slots, F2], BF16, tag="xw2s", bufs=1)

            qa = tuple(range(n_queues // 2))
            qb = tuple(range(n_queues // 2, n_queues))

            skip_g = mode == "no_gather"
            skip_mm = mode == "gather_only"

            if skip_mm or io_only:
                zo = wpool.tile([P, 2 * P], BF16, tag="outz", bufs=1)
                nc.vector.memset(zo[:], 0.0)
                z2 = wpool.tile([P, F2], BF16, tag="m2z", bufs=1)
                nc.vector.memset(z2[:], 0.0)
            if skip_mm:
                nc.vector.memset(ySelf[:], 0.0)

            if io_only:
                for m_ in range(npairs):
                    nc.sync.dma_start(out[:, m_ * 2 * P:(m_ + 1) * 2 * P],
                                      zo[:])

            for _rep in range(repeat):
              if _rep > 0:
                  # full barrier so R-diff timing measures serial per-pass
                  # time (matches back-to-back single executions)
                  tc.strict_bb_all_engine_barrier()

              # ---- stage 1: y = x~ @ W1 (fp8) + publish via 2 AllGathers --
              if not io_only:
                for j in range(slots):
                    xsl = wpool.tile([P, 2, P], BF16, tag="xsl", bufs=3)
                    nc.sync.dma_start(xsl[:], xT[:, :, j * P:(j + 1) * P])
                    if not skip_mm:
                        py = ppool.tile([P, F1], F32, tag="agg", bufs=agg_bufs)
                        for k in range(2):
                            nc.tensor.matmul(py[:], lhsT=xsl[:, k, :],
                                             rhs=w1_t[:, k, :],
                                             start=(k == 0), stop=(k == 1))
                        nc.scalar.activation(ySelf[:, j, :], py[:], ACT.Copy)
                    if j < SA:
                        nc.sync.dma_start(ysh_a[j * P:(j + 1) * P, :],
                                          ySelf[:, j, :])
                        if j == SA - 1:
                            fire_ag(ysh_a, xs_a)
                    else:
                        jb = j - SA
                        nc.sync.dma_start(ysh_b[jb * P:(jb + 1) * P, :],
                                          ySelf[:, j, :])
                        if j == slots - 1:
                            fire_ag(ysh_b, xs_b)

              st1 = [
                  GatherStream(nc, mpool, xs_a[:, :], i1a_t, meta["tot1"][0],
                               F1, "m1a", slab_chunks, bufs=gbufs1a,
                               queues=qa, dt=FP8),
                  GatherStream(nc, mpool, xs_b[:, :], i1b_t, meta["tot1"][1],
                               F1, "m1b", slab_chunks, bufs=gbufs,
                               queues=qb, dt=FP8),
              ]
              st2 = [
                  GatherStream(nc, mpool, t2_a[:, :], i2a_t, meta["tot2"][0],
                               F2, "m2a", slab_chunks, bufs=gbufs2a,
                               queues=qa),
                  GatherStream(nc, mpool, t2_b[:, :], i2b_t, meta["tot2"][1],
                               F2, "m2b", slab_chunks, bufs=gbufs,
                               queues=qb),
              ]

              # ---- stage 2: layer-1 aggregation, pipelined epilogue ----
              def agg_l1(j, ck):
                  nch = int(K1[j, 0] + K1[j, 1])
                  if skip_mm:
                      for h in range(2):
                          for _ in range(int(K1[j, h])):
                              st1[h].next_chunk()
                      return None, ck + nch
                  psum = ppool.tile([P, F1], F32, tag="agg", bufs=agg_bufs)
                  ci = 0
                  for h in range(2):
                      left = int(K1[j, h])
                      while left > 0:
                          nb = min(oh_batch, left)
                          oh = wpool.tile([P, oh_batch, P], FP8, tag="oh",
                                          bufs=2)
                          nc.vector.tensor_tensor(
                              out=oh[:, :nb, :],
                              in0=cs1_t[:, ck:ck + nb, None]
                                  .to_broadcast([P, nb, P]),
                              in1=io1_t[:, :nb * P],
                              op=AL.is_equal,
                          )
                          i = 0
                          while i < nb:
                              if (double_row and i + 1 < nb
                                      and st1[h].can_pair()):
                                  msg2 = st1[h].next_chunk2(skip_g)
                                  nc.tensor.matmul(
                                      psum[:], lhsT=oh[:, i:i + 2, :],
                                      rhs=msg2, start=(ci == 0), stop=False,
                                      perf_mode=DR)
                                  i += 2
                                  ci += 2
                              else:
                                  msg = st1[h].next_chunk(skip_g)
                                  nc.tensor.matmul(
                                      psum[:], lhsT=oh[:, i, :], rhs=msg,
                                      start=(ci == 0), stop=False)
                                  i += 1
                                  ci += 1
                          ck += nb
                          left -= nb
                  nc.tensor.matmul(psum[:], lhsT=id8_t[:], rhs=ySelf[:, j, :],
                                   start=(nch == 0), stop=True)
                  return psum, ck

              def epi_l1(j, psum):
                  if psum is None:
                      m2t = z2[:]
                  else:
                      htile = wpool.tile([P, F1], BF16, tag="h", bufs=2)
                      if b1_zero:
                          nc.scalar.activation(htile[:], psum[:], ACT.Relu,
                                               scale=dt_t[:, j:j + 1])
                      else:
                          t1 = wpool.tile([P, F1], F32, tag="ep1", bufs=2)
                          nc.vector.tensor_scalar(t1[:], psum[:],
                                                  dt_t[:, j:j + 1], None,
                                                  op0=AL.mult)
                          nc.vector.tensor_tensor(t1[:], t1[:], b1_t[:],
                                                  op=AL.add)
                          nc.scalar.activation(htile[:], t1[:], ACT.Relu)
                      hT = wpool.tile([P, 2, P], BF16, tag="hT", bufs=2)
                      for k in range(2):
                          ptr = ppool.tile([P, P], BF16, tag="tr", bufs=2)
                          nc.tensor.transpose(ptr[:],
                                              htile[:, k * P:(k + 1) * P],
                                              id_t[:])
                          nc.scalar.activation(hT[:, k, :], ptr[:], ACT.Copy)
                      pd2 = ppool.tile([P, F2], F32, tag="dense", bufs=2)
                      for k in range(2):
                          nc.tensor.matmul(pd2[:], lhsT=hT[:, k, :],
                                           rhs=w2_t[:, k, :],
                                           start=(k == 0), stop=(k == 1))
                      nc.scalar.activation(xw2s[:, j, :], pd2[:], ACT.Copy,
                                           scale=dt_t[:, j:j + 1])
                      m2t = xw2s[:, j, :]
                  if j < SA:
                      nc.sync.dma_start(m2h_a[j * P:(j + 1) * P, :], m2t)
                      if j == SA - 1:
                          fire_ag(m2h_a, t2_a)
                  else:
                      jb = j - SA
                      nc.sync.dma_start(m2h_b[jb * P:(jb + 1) * P, :], m2t)
                      if j == slots - 1:
                          fire_ag(m2h_b, t2_b)

              ck1 = 0
              prev = None
              for j in range(slots if not io_only else 0):
                  psum, ck1 = agg_l1(j, ck1)
                  if prev is not None:
                      epi_l1(*prev)
                  prev = (j, psum) if pipe else None
                  if not pipe:
                      epi_l1(j, psum)
              if prev is not None:
                  epi_l1(*prev)

              # ---- stage 3: layer-2 aggregation + epilogue ----
              def agg_l2(m_, ck):
                  nch = int(K2[m_, 0] + K2[m_, 1])
                  if skip_mm:
                      for h in range(2):
                          for _ in range(int(K2[m_, h])):
                              st2[h].next_chunk()
                      return None, ck + nch
                  psumT = ppool.tile([P, 2 * P], F32, tag="agg",
                                     bufs=agg_bufs)
                  ci = 0
                  for h in range(2):
                      left = int(K2[m_, h])
                      while left > 0:
                          nb = min(OB2, left)
                          moh = wpool.tile([P, OB2, 2 * P], BF16, tag="moh",
                                           bufs=2)
                          nc.vector.tensor_tensor(
                              out=moh[:, :nb, :],
                              in0=cs2_t[:, ck:ck + nb, None]
                                  .to_broadcast([P, nb, 2 * P]),
                              in1=io2_t[:, :nb * 2 * P],
                              op=AL.is_equal,
                          )
                          for i in range(nb):
                              msg = st2[h].next_chunk(skip_g)
                              nc.tensor.matmul(psumT[:], lhsT=msg,
                                               rhs=moh[:, i, :],
                                               start=(ci == 0), stop=False)
                              ci += 1
                          ck += nb
                          left -= nb
                  nsl = min(2, slots - 2 * m_)
                  for k in range(nsl):
                      nc.tensor.matmul(psumT[:, k * P:(k + 1) * P],
                                       lhsT=xw2s[:, 2 * m_ + k, :],
                                       rhs=id_t[:],
                                       start=(nch == 0 and k == 0),
                                       stop=(k == nsl - 1))
                  return psumT, ck

              def epi_l2(m_, psumT):
                  if psumT is None:
                      nc.sync.dma_start(out[:, m_ * 2 * P:(m_ + 1) * 2 * P],
                                        zo[:])
                      return
                  t2 = wpool.tile([P, 2 * P], BF16, tag="ep2", bufs=2)
                  if b2_zero:
                      nc.vector.tensor_tensor(
                          t2[:], psumT[:],
                          d2_t[:, m_ * 2 * P:(m_ + 1) * 2 * P], op=AL.mult)
                  else:
                      tf = wpool.tile([P, 2 * P], F32, tag="ep2f", bufs=2)
                      nc.vector.tensor_tensor(
                          tf[:], psumT[:],
                          d2_t[:, m_ * 2 * P:(m_ + 1) * 2 * P], op=AL.mult)
                      nc.vector.tensor_scalar(t2[:], tf[:], b2_t[:, 0:1],
                                              None, op0=AL.add)
                  nc.sync.dma_start(out[:, m_ * 2 * P:(m_ + 1) * 2 * P],
                                    t2[:])

              ck2 = 0
              prev2 = None
              for m_ in range(npairs if not io_only else 0):
                  psumT, ck2 = agg_l2(m_, ck2)
                  if prev2 is not None:
                      epi_l2(*prev2)
                  prev2 = (m_, psumT) if pipe else None
                  if not pipe:
                      epi_l2(m_, psumT)
              if prev2 is not None:
                  epi_l2(*prev2)

    nc.compile()
    return nc


class SpmdRunner:
    def __init__(self, nc, n_cores: int = 8, nreps: int = 1,
                 tick_name: str = "tick", tock_name: str = "tock"):
        install_neuronx_cc_hook()
        self.nc = nc
        self.n_cores = n_cores
        assert nc.dbg_addr is None or not nc.dbg_callbacks
        self.dbg_name = nc.dbg_addr.name if nc.dbg_addr is not None else None
        partition_name = nc.partition_id_tensor.name if nc.partition_id_tensor else None

        in_names, out_names, out_avals = [], [], []
        for alloc in nc.m.functions[0].allocations:
            if not isinstance(alloc, mybir.MemoryLocationSet):
                continue
            name = alloc.memorylocations[0].name
            if alloc.kind == "ExternalInput":
                if name != partition_name:
                    in_names.append(name)
            elif alloc.kind == "ExternalOutput":
                out_names.append(name)
                shape = tuple(alloc.tensor_shape)
                dtype = mybir.dt.np(alloc.dtype)
                out_avals.append(jax.core.ShapedArray(shape, dtype))
        self.in_names = in_names      # order matters; includes dbg if declared
        self.out_names = out_names
        self.out_avals = out_avals
        n_params = len(in_names)
        n_outs = len(out_avals)
        all_in_names = list(in_names) + list(out_names)
        if partition_name is not None:
            all_in_names.append(partition_name)

        tick_i = in_names.index(tick_name) if (nreps > 1 and tick_name in in_names) else None
        tock_i = out_names.index(tock_name) if (nreps > 1 and tock_name in out_names) else None
        assert nreps == 1 or (tick_i is not None and tock_i is not None), \
            "nreps>1 needs tick/tock passthrough tensors in the kernel"

        def _call(operands):
            if partition_name is not None:
                operands = operands + [partition_id_tensor()]
            return _bass_exec_p.bind(
                *operands,
                out_avals=tuple(out_avals),
                in_names=tuple(all_in_names),
                out_names=tuple(out_names),
                lowering_input_output_aliases=(),
                sim_require_finite=True,
                sim_require_nnan=True,
                nc=nc,
            )

        def _body(*args):
            operands = list(args)
            outs = _call(list(operands))
            for _ in range(nreps - 1):
                operands2 = list(operands)
                operands2[tick_i] = outs[tock_i]
                outs = _call(operands2)
            return tuple(outs)

        devices = jax.devices()[: self.n_cores]
        assert len(devices) == self.n_cores
        mesh = Mesh(np.asarray(devices), ("core",))
        self._sharding = jax.sharding.NamedSharding(mesh, PartitionSpec("core"))
        in_specs = (PartitionSpec("core"),) * (n_params + n_outs)
        out_specs = (PartitionSpec("core"),) * n_outs
        self._fn = jax.jit(
            shard_map(_body, mesh=mesh, in_specs=in_specs, out_specs=out_specs,
                      check_rep=False),
            keep_unused=True,
        )
        # output operand buffers are created ON DEVICE (no host staging)
        zero_shapes = [(self.n_cores * z.shape[0], *z.shape[1:])
                       for z in out_avals]
        zero_dtypes = [a.dtype for a in out_avals]

        def _mk_zeros():
            return tuple(jnp.zeros(s, d) for s, d in
                         zip(zero_shapes, zero_dtypes))

        self._zeros_fn = jax.jit(
            _mk_zeros, out_shardings=(self._sharding,) * n_outs)
        self._dev_zeros = None
        self._dev_in = None

    def stage_inputs(self, in_maps):
        """in_maps: list (len n_cores) of dict name->np.ndarray."""
        if self.dbg_name is not None:
            in_maps = [
                {**m, self.dbg_name: np.zeros((1, 2), np.uint32)} for m in in_maps
            ]
        concat_in = [
            np.concatenate([np.asarray(in_maps[c][name]) for c in range(self.n_cores)],
                           axis=0)
            for name in self.in_names
        ]
        self._dev_in = [jax.device_put(a, self._sharding) for a in concat_in]
        self._dev_zeros = list(self._zeros_fn())
        jax.block_until_ready(self._dev_in)
        jax.block_until_ready(self._dev_zeros)

    def run(self):
        outs = self._fn(*self._dev_in, *self._dev_zeros)
        jax.block_until_ready(outs)
        return outs

    def run_chain(self, n):
        """Dispatch n executions back-to-back (tick chained through tock to
        force strict ordering), block once at the end."""
        ti = self.in_names.index("tick")
        oi = self.out_names.index("tock")
        ins = list(self._dev_in)
        outs = self._fn(*ins, *self._dev_zeros)
        for _ in range(n - 1):
            ins[ti] = outs[oi]
            outs = self._fn(*ins, *self._dev_zeros)
        jax.block_until_ready(outs)
        return outs

    def results(self, outs):
        return [
            {
                name: np.asarray(outs[i]).reshape(self.n_cores, *self.out_avals[i].shape)[c]
                for i, name in enumerate(self.out_names)
            }
            for c in range(self.n_cores)
        ]


# ----------------------------------------------------------------------------
# Public entry point
# ----------------------------------------------------------------------------

_CACHE = {}

BUILD_KW = dict(slab_chunks=16, oh_batch=16, agg_bufs=4,
                gbufs=4, gbufs1a=7, gbufs2a=5)


def kernel(**inputs) -> np.ndarray:
    x = np.asarray(inputs["x"], np.float32)
    edge_index = np.asarray(inputs["edge_index"], np.int64)
    W1 = np.asarray(inputs["W1"], np.float32)
    b1 = np.asarray(inputs["b1"], np.float32)
    W2 = np.asarray(inputs["W2"], np.float32)
    b2 = np.asarray(inputs["b2"], np.float32)

    in_maps, meta = plan_host(x, edge_index, W1, b1, W2, b2)
    for m in in_maps:
        m["tick"] = np.zeros((1, 4), np.float32)

    key = (x.shape, edge_index.shape, W2.shape,
           tuple(meta["K1"].reshape(-1)), tuple(meta["K2"].reshape(-1)),
           meta["b1_zero"], meta["b2_zero"])
    if key not in _CACHE:
        nc = build_nc(meta, **BUILD_KW)
        _CACHE[key] = SpmdRunner(nc, meta["n_cores"])
    runner = _CACHE[key]
    runner.stage_inputs(in_maps)
    outs = runner.run()
    res = runner.results(outs)
    shards = [res[c]["out"] for c in range(meta["n_cores"])]
    return assemble_output(shards, meta).astype(np.float32)
